# revision 1
# baseline (speedup 1.0000x reference)
"""CornerPool2d (TopLeft): out = suffix_cummax_H(x) + suffix_cummax_W(x).

Input: [16, 256, 256, 64] f32. Sharded batch-wise across 8 NeuronCores
(2 images per core). Per core, two passes:
  Pass A (per 128-wide w-tile): load x as [W_p, (h, c)], blocked suffix-max
    over h on DVE, store h_pool to a DRAM scratch tensor.
  Pass B (per 128-wide h-tile): load x as [H_p, (w, c)] (contiguous),
    blocked suffix-max over w, reload h_pool in the same layout, add, store.
"""

import numpy as np

import concourse.bacc as bacc
import concourse.mybir as mybir
import concourse.tile as tile

F32 = mybir.dt.float32

B_FULL = 16
N_CORES = 8
B = B_FULL // N_CORES  # images per core
H = 256
W = 256
C = 64
P = 128

_cache = {}


def _suffix_scan_free(nc, small_pool, t_ap, axis_len, b, C):
    """In-place suffix (reverse cumulative) max along the middle axis of t_ap [P, axis_len, C]."""
    nb = axis_len // b
    assert nb * b == axis_len
    v = t_ap.rearrange("p (j i) c -> p j i c", i=b)
    for i in range(b - 2, -1, -1):
        nc.vector.tensor_max(out=v[:, :, i, :], in0=v[:, :, i, :], in1=v[:, :, i + 1, :])
    if nb == 1:
        return
    p = t_ap.shape[0]
    s = small_pool.tile([p, nb - 1, C], F32, tag="s")
    nc.vector.tensor_copy(out=s[:, nb - 2, :], in_=v[:, nb - 1, 0, :])
    for j in range(nb - 3, -1, -1):
        nc.vector.tensor_max(out=s[:, j, :], in0=s[:, j + 1, :], in1=v[:, j + 1, 0, :])
    for i in range(b):
        nc.vector.tensor_max(out=v[:, 0 : nb - 1, i, :], in0=v[:, 0 : nb - 1, i, :], in1=s[:])


def _build_program(b_h=16, b_w=16):
    nc = bacc.Bacc("TRN2", target_bir_lowering=False, debug=False)
    x = nc.dram_tensor("x", [B, H, W, C], F32, kind="ExternalInput")
    y = nc.dram_tensor("y", [B, H, W, C], F32, kind="ExternalOutput")
    with tile.TileContext(nc) as tc:
        with (
            tc.tile_pool(name="dram", bufs=1, space="DRAM") as dpool,
            tc.tile_pool(name="big", bufs=3) as big,
            tc.tile_pool(name="small", bufs=4) as small,
        ):
            hp = dpool.tile([B, H, W, C], F32)
            # loads on the SP HWDGE ring, stores on the ACT HWDGE ring so
            # they drain through independent FIFOs and overlap.
            ld, st = nc.sync, nc.scalar
            for img in range(B):
                for wb in range(W // P):
                    a = big.tile([P, H, C], F32, tag="big")
                    src = x[img].rearrange("h w c -> w h c")[wb * P : (wb + 1) * P]
                    ld.dma_start(out=a[:], in_=src)
                    _suffix_scan_free(nc, small, a[:], H, b_h, C)
                    dst = hp[img].rearrange("h w c -> w h c")[wb * P : (wb + 1) * P]
                    st.dma_start(out=dst, in_=a[:])
            for img in range(B):
                for hb in range(H // P):
                    bx = big.tile([P, W, C], F32, tag="big")
                    ld.dma_start(out=bx[:], in_=x[img, hb * P : (hb + 1) * P])
                    _suffix_scan_free(nc, small, bx[:], W, b_w, C)
                    hpt = big.tile([P, W, C], F32, tag="big")
                    ld.dma_start(out=hpt[:], in_=hp[img, hb * P : (hb + 1) * P])
                    nc.vector.tensor_add(out=bx[:], in0=bx[:], in1=hpt[:])
                    st.dma_start(out=y[img, hb * P : (hb + 1) * P], in_=bx[:])
    nc.compile()
    return nc


def kernel(inputs: np.ndarray) -> np.ndarray:
    """Graded entry point — proven run_bass_kernel_spmd path."""
    from concourse import bass_utils

    x = np.ascontiguousarray(np.asarray(inputs, dtype=np.float32))
    assert x.shape == (B_FULL, H, W, C), x.shape
    if "nc" not in _cache:
        _cache["nc"] = _build_program()
    nc = _cache["nc"]
    in_maps = [{"x": x[i * B : (i + 1) * B]} for i in range(N_CORES)]
    res = bass_utils.run_bass_kernel_spmd(nc, in_maps, core_ids=list(range(N_CORES)))
    return np.concatenate([r["y"] for r in res.results], axis=0)

